# revision 1
# baseline (speedup 1.0000x reference)
"""Trainium2 Bass kernel for nn_AttentionWPooling.

Math (per batch b):
  a = x0[b,0], bb = x1[b,0]                       # [S, H], S=2050, H=128
  d2[i,j] = |a_i|^2 + |b_j|^2 - 2 a_i.b_j
  A[i,j]  = 1 / (1 + sqrt(d2))
  r[j] = sum_i A[i,j]; c[i] = sum_j A[i,j]
  w0 = r[:,None]*a ; w1 = c[:,None]*bb
  wp{0,1}[j] = sum_{k=j..j+2} w{0,1}[k]           # j in [0, 2048)

Device mapping: data-parallel over batch, 4 batches per core on 8 cores.

Fast path (custom_act=True, default): the compiler's activation-table
dir is copied and the Sqrt spline payloads are refit to
g(x) = 1/(1+sqrt(x)), so ONE ScalarE pass computes A directly from the
matmul PSUM (scale=-2, bias=|a_i|^2; PSUM holds cross - |b_j|^2/2 via a
K=2 augmented matmul with bf16 hi/lo -sq1/2 rows).  Row sums c ride the
activation accumulator; column sums r = ones-matmul over an fp16 racc
accumulated on VectorE.  Windowed pooling = banded matmul with constant
band tiles.  Fallback path (custom_act=False): A = Sigmoid(-0.5*Ln(d2))
in two ScalarE passes (exact identity), same everything else.
"""

import functools
import os

import numpy as np
import ml_dtypes

import concourse.bass as bass
from concourse import bacc
import concourse.mybir as mybir
import concourse.tile as tile
from concourse.bass import ts
from concourse.bass_utils import run_bass_kernel_spmd

F32 = mybir.dt.float32
BF16 = mybir.dt.bfloat16
FP16 = mybir.dt.float16
AF = mybir.ActivationFunctionType

N_CORES = 8
B_TOTAL = 32
B_PER_CORE = B_TOTAL // N_CORES  # 4
S = 2050
H = 128
NT = 17            # i-tiles of 128 rows (17*128 = 2176)
SPAD = NT * 128    # 2176, padded S (j padded with huge distances)
L_OUT = 2048
PAD_SQ1 = 1e16     # padded |b_j|^2 -> dist ~ 1e8 -> A ~ 1e-8 ~ 0
                   # (must stay below 2^64: Ln's valid input range)
JPAD = S  # exact j range: no padded columns
JCH = ((0, 1024), (1024, 1026))  # j-chunks; psum tiles of 2 and 3 banks


def _gen_custom_act_dir():
    """Build an act-table dir where Sqrt's spline is replaced by
    g(x) = 1/(1+sqrt(x)), so one ScalarE pass computes A from d2.

    Patches only the bucket payloads of func 'sqrt' inside the
    'sqrt_and_others' set; profile/ctrl tables (section structure,
    exponent binning) are unchanged.
    """
    import json
    import shutil
    import tempfile

    from neuronxcc.driver.Job import Job
    from neuronxcc.driver.jobs.support.FindActInfo import findActInfoFile

    act_info_path = findActInfoFile(Job.getPackageDir(), "gen3")
    src_dir = os.path.dirname(act_info_path)
    pwp_json = os.path.join(src_dir, "..", "pwp_jsons", "sqrt_65536p.json")
    spec = json.load(open(pwp_json))
    meta = json.load(open(os.path.join(src_dir, "sqrt_and_others.json")))
    start = meta["func_to_bkt_start_idx"]["sqrt"]

    def g(x):
        return 1.0 / (1.0 + np.sqrt(x))

    recs = []
    for e in spec["pos_exponents"]:
        eb, es = e["exponent"], e["extract_size"]
        width = 2.0 ** eb
        for si, s in enumerate(e["exponent_sections"]):
            x0 = (
                np.frombuffer(np.uint32(s["x"]["int"]).tobytes(), np.float32)[0]
                .item()
            )
            lo = width * (1.0 + si / (1 << es))
            hi = width * (1.0 + (si + 1) / (1 << es))
            xs = np.linspace(lo, hi, 64, dtype=np.float64)
            tt = xs - x0
            yy = g(xs)
            c32 = None
            for deg in (3, 1, 0):
                w = 1.0 / np.abs(yy)
                V = np.vander(tt, deg + 1, increasing=True) * w[:, None]
                coef, *_ = np.linalg.lstsq(V, yy * w, rcond=None)
                cc = np.zeros(4)
                cc[: deg + 1] = coef
                cand = cc.astype(np.float32)
                if not np.all(np.isfinite(cand)):
                    continue
                t32 = tt.astype(np.float32)
                y32 = cand[0] + t32 * (cand[1] + t32 * (cand[2] + t32 * cand[3]))
                rel = np.max(np.abs(y32 - yy) / np.abs(yy))
                if rel < 1e-4 or deg == 0:
                    c32 = cand
                    break
            if c32 is None:
                c32 = np.array([yy.mean(), 0, 0, 0], np.float32)
            recs.append((c32, np.float32(x0)))

    dst = tempfile.mkdtemp(prefix="actpatch_")
    for f in os.listdir(src_dir):
        shutil.copy(os.path.join(src_dir, f), os.path.join(dst, f))
    binpath = os.path.join(dst, "sqrt_and_others_bkt.bin")
    arr = np.frombuffer(open(binpath, "rb").read(), np.uint32).copy()
    for k, (c32, x0) in enumerate(recs):
        base = (start + k) * 8
        arr[base : base + 4] = c32.view(np.uint32)
        arr[base + 4] = np.float32(x0).view(np.uint32)
    open(binpath, "wb").write(arr.tobytes())
    return dst


def _make_bands():
    # WT[k, j] = 1 iff the window of output j covers row k:  j <= k <= j+2.
    band0 = np.zeros((128, 128), np.float32)
    band1 = np.zeros((128, 128), np.float32)
    for k in range(128):
        for j in range(128):
            if 0 <= k - j <= 2:
                band0[k, j] = 1.0
            # band1: rows k of the NEXT k-tile: 1 iff j <= k+128 <= j+2
            if 0 <= (k + 128) - j <= 2:
                band1[k, j] = 1.0
    return band0, band1


USE_CUSTOM_ACT = os.environ.get("KERNEL_CUSTOM_ACT", "1") == "1"


def _build(b_per_core=B_PER_CORE, custom_act=None):
    if custom_act is None:
        custom_act = USE_CUSTOM_ACT
    if custom_act:
        try:
            actdir = _gen_custom_act_dir()
            os.environ["BASS_ACT_ROOT_JSON_PATH"] = os.path.join(
                actdir, "act_info.json"
            )
        except Exception:
            custom_act = False  # fall back to Sigmoid(-0.5*Ln(d2)) path
    nc = bacc.Bacc("TRN2", target_bir_lowering=False)
    B = b_per_core

    x0 = nc.dram_tensor("x0", [B, S, H], F32, kind="ExternalInput")
    x1 = nc.dram_tensor("x1", [B, S, H], F32, kind="ExternalInput")
    # host-precomputed row norms: sq0n[b, p, t] = |a_{128t+p}|^2  (0 past S)
    sq0n = nc.dram_tensor("sq0n", [B, 128, NT], F32, kind="ExternalInput")
    # host-precomputed -0.5*|b_j|^2 padded with -0.5*PAD_SQ1, bf16 hi/lo rows
    sq1hl = nc.dram_tensor("sq1hl", [B, 2, JPAD], BF16, kind="ExternalInput")

    o0 = nc.dram_tensor("o0", [B, L_OUT, H], F32, kind="ExternalOutput")
    o1 = nc.dram_tensor("o1", [B, L_OUT, H], F32, kind="ExternalOutput")

    ident_bf = nc.inline_tensor(np.eye(128, dtype=ml_dtypes.bfloat16), "identbf")
    ones2 = nc.inline_tensor(np.ones((2, 128), dtype=ml_dtypes.bfloat16), "ones2")
    ones128 = nc.inline_tensor(np.ones((128, 1), dtype=np.float16), "ones128")
    b0np, b1np = _make_bands()
    band0 = nc.inline_tensor(b0np.astype(np.float16), "band0")
    band1 = nc.inline_tensor(b1np.astype(np.float16), "band1")

    with tile.TileContext(nc) as tc:
        with (
            tc.tile_pool(name="pin", bufs=2) as pin,
            tc.tile_pool(name="p16", bufs=1) as p16,
            tc.tile_pool(name="pT", bufs=2) as pT,
            tc.tile_pool(name="pbig", bufs=1) as pbig,
            tc.tile_pool(name="pacc", bufs=2) as pacc,
            tc.tile_pool(name="psmall", bufs=2) as psmall,
            tc.tile_pool(name="pw", bufs=2 if custom_act else 1) as pw,
            tc.tile_pool(name="posb", bufs=2 if custom_act else 1) as posb,
            tc.tile_pool(name="ppsA", bufs=2, space="PSUM") as ppsA,
            tc.tile_pool(name="ppsT", bufs=1, space="PSUM") as ppsT,
            tc.tile_pool(name="ppsM", bufs=2, space="PSUM") as ppsM,
        ):
            idsb = psmall.tile([128, 128], BF16, tag="idsb", bufs=1)
            nc.sync.dma_start(out=idsb, in_=ident_bf[:, :])
            ones2sb = psmall.tile([2, 128], BF16, tag="ones2", bufs=1)
            nc.sync.dma_start(out=ones2sb, in_=ones2[:, :])
            ones128sb = psmall.tile([128, 1], FP16, tag="ones128", bufs=1)
            nc.sync.dma_start(out=ones128sb, in_=ones128[:, :])
            band0sb = psmall.tile([128, 128], FP16, tag="band0", bufs=1)
            nc.sync.dma_start(out=band0sb, in_=band0[:, :])
            band1sb = psmall.tile([128, 128], FP16, tag="band1", bufs=1)
            nc.sync.dma_start(out=band1sb, in_=band1[:, :])
            onef32sb = psmall.tile([1, 1], F32, tag="onef32", bufs=1)
            nc.vector.memset(onef32sb, 1.0)

            state = [None] * B

            def emit_pm(b):
                """Prologue (loads, casts, transposes) + main A-loop."""
                # ---- load inputs (natural layout, zero-padded tail tile) ---
                anat = pin.tile([128, NT, 128], F32, tag="anat")
                bnat = pin.tile([128, NT, 128], F32, tag="bnat")
                nc.vector.memset(anat[:, NT - 1, :], 0.0)
                nc.vector.memset(bnat[:, NT - 1, :], 0.0)
                nc.sync.dma_start(
                    out=anat[:, : NT - 1, :],
                    in_=x0[b, : (NT - 1) * 128].rearrange(
                        "(t p) h -> p t h", p=128
                    ),
                )
                nc.sync.dma_start(
                    out=anat[:2, NT - 1 : NT, :],
                    in_=x0[b, (NT - 1) * 128 : S].rearrange(
                        "(t p) h -> p t h", p=2
                    ),
                )
                nc.sync.dma_start(
                    out=bnat[:, : NT - 1, :],
                    in_=x1[b, : (NT - 1) * 128].rearrange(
                        "(t p) h -> p t h", p=128
                    ),
                )
                nc.sync.dma_start(
                    out=bnat[:2, NT - 1 : NT, :],
                    in_=x1[b, (NT - 1) * 128 : S].rearrange(
                        "(t p) h -> p t h", p=2
                    ),
                )
                sq0sb = psmall.tile([128, NT], F32, tag="sq0")
                nc.sync.dma_start(out=sq0sb, in_=sq0n[b])
                sq1sb = psmall.tile([2, JPAD], BF16, tag="sq1")
                nc.sync.dma_start(out=sq1sb, in_=sq1hl[b])

                # ---- bf16 casts + PE transposes -> aT16/bT16 [128h, SPAD] --
                a16 = p16.tile([128, NT, 128], BF16, tag="a16")
                b16 = p16.tile([128, NT, 128], BF16, tag="b16")
                nc.vector.tensor_copy(a16, anat)
                nc.vector.tensor_copy(b16, bnat)
                aT16 = pT.tile([128, SPAD], BF16, tag="aT16")
                bT16 = pT.tile([128, SPAD], BF16, tag="bT16")
                for src, dst in ((a16, aT16), (b16, bT16)):
                    for g0 in range(0, NT, 8):
                        glen = min(8, NT - g0)
                        psT = ppsT.tile([128, 8, 128], BF16, tag="tp")
                        for k in range(glen):
                            nc.tensor.transpose(
                                psT[:, k, :], src[:, g0 + k, :], idsb
                            )
                        nc.vector.tensor_copy(
                            dst[:, g0 * 128 : (g0 + glen) * 128],
                            psT[:, :glen, :],
                        )

                racc = pacc.tile([128, JPAD], FP16, tag="racc")
                cnat = pacc.tile([128, NT], F32, tag="cnat")
                nc.vector.memset(racc, 0.0)

                def mm_chunk(ps, t, jo, jw):
                    for s0 in range(0, jw, 512):
                        sw = min(512, jw - s0)
                        nc.tensor.matmul(
                            ps[:, s0 : s0 + sw],
                            lhsT=aT16[:, ts(t, 128)],
                            rhs=bT16[:, jo + s0 : jo + s0 + sw],
                            start=True,
                            stop=False,
                        )
                        nc.tensor.matmul(
                            ps[:, s0 : s0 + sw],
                            lhsT=ones2sb,
                            rhs=sq1sb[:, jo + s0 : jo + s0 + sw],
                            start=False,
                            stop=True,
                        )

                c3 = None
                if custom_act:
                    # ---- single pass: A = g(d2) via patched Sqrt table -----
                    c3 = pacc.tile([128, NT, 2], F32, tag="c3")
                    for t in range(NT):
                        plim = 128 if t < NT - 1 else (S - (NT - 1) * 128)
                        for ci, (jo, jw) in enumerate(JCH):
                            ps = ppsA.tile(
                                [128, jw], F32, tag=f"mm{ci}", bufs=1
                            )
                            mm_chunk(ps, t, jo, jw)
                            At = psmall.tile(
                                [128, jw], FP16, tag=f"At{ci}", bufs=4
                            )
                            nc.scalar.activation(
                                out=At,
                                in_=ps,
                                func=AF.Sqrt,  # patched: 1/(1+sqrt(x))
                                bias=sq0sb[:, t : t + 1],
                                scale=-2.0,
                                accum_out=c3[:, t, ci : ci + 1],
                            )
                            nc.vector.tensor_add(
                                racc[:plim, jo : jo + jw],
                                racc[:plim, jo : jo + jw],
                                At[:plim, :],
                            )
                else:
                    # ---- two-pass fallback: A = Sigmoid(-0.5*Ln(d2)) -------
                    Lbuf = pbig.tile([128, NT, JPAD], FP16, tag="L")
                    for t in range(NT):
                        for jo, jw in JCH:
                            ps = ppsA.tile(
                                [128, jw], F32, tag=f"mm{jw}", bufs=1
                            )
                            mm_chunk(ps, t, jo, jw)
                            nc.scalar.activation(
                                out=Lbuf[:, t, jo : jo + jw],
                                in_=ps,
                                func=AF.Ln,
                                bias=sq0sb[:, t : t + 1],
                                scale=-2.0,
                            )
                    tc.no_sync_barrier()
                    for t in range(NT):
                        At = psmall.tile([128, JPAD], FP16, tag="Atf")
                        nc.scalar.activation(
                            out=At,
                            in_=Lbuf[:, t, :],
                            func=AF.Sigmoid,
                            scale=-0.5,
                            accum_out=cnat[:, t : t + 1],
                        )
                        plim = 128 if t < NT - 1 else (S - (NT - 1) * 128)
                        nc.vector.tensor_add(
                            racc[:plim, :], racc[:plim, :], At[:plim, :]
                        )
                    tc.no_sync_barrier()

                state[b] = dict(
                    anat=anat, bnat=bnat, racc=racc, cnat=cnat, c3=c3
                )

            def emit_epi(b):
                """r reduction + w tensors + banded pooling + output DMA."""
                st = state[b]
                anat, bnat = st["anat"], st["bnat"]
                racc, cnat, c3 = st["racc"], st["cnat"], st["c3"]
                if c3 is not None:
                    nc.vector.reduce_sum(cnat, c3, axis=mybir.AxisListType.X)

                # ---- r = partition-sum of racc via ones-matmul -------------
                rfree = psmall.tile([1, JPAD], F32, tag="rfree")
                for jo in range(0, JPAD, 512):
                    jw = min(512, JPAD - jo)
                    rps = ppsM.tile([128, 512], F32, tag="misc")
                    nc.tensor.matmul(
                        rps[:1, :jw],
                        lhsT=ones128sb,
                        rhs=racc[:, jo : jo + jw],
                        start=True,
                        stop=True,
                    )
                    nc.vector.tensor_copy(rfree[:, jo : jo + jw], rps[:1, :jw])
                # scatter r to per-partition layout via K=1 matmuls
                rnps = ppsM.tile([128, 512], F32, tag="misc")
                for t in range(NT):
                    tw = min(128, JPAD - 128 * t)
                    nc.tensor.matmul(
                        rnps[:tw, t : t + 1],
                        lhsT=rfree[:, 128 * t : 128 * t + tw],
                        rhs=onef32sb,
                        start=True,
                        stop=True,
                    )
                rnat = psmall.tile([128, NT], F32, tag="rnat")
                nc.vector.memset(rnat[:, NT - 1 :], 0.0)
                nc.vector.tensor_copy(rnat[:, : NT - 1], rnps[:, : NT - 1])
                nc.vector.tensor_copy(
                    rnat[: JPAD - 128 * (NT - 1), NT - 1 : NT],
                    rnps[: JPAD - 128 * (NT - 1), NT - 1 : NT],
                )

                # ---- w0 = r*a, w1 = c*b (fp16) -----------------------------
                w0f = pw.tile([128, NT, 128], FP16, tag="w0")
                w1f = pw.tile([128, NT, 128], FP16, tag="w1")
                # emit the first pool-group's w tiles (0:5) as a separate
                # small op so PE pooling starts while V finishes the rest
                for _h0, _h1 in ((0, 5), (5, NT)):
                    nc.vector.tensor_tensor(
                        w0f[:, _h0:_h1, :],
                        anat[:, _h0:_h1, :],
                        rnat[:, _h0:_h1, None].to_broadcast(
                            (128, _h1 - _h0, 128)
                        ),
                        mybir.AluOpType.mult,
                    )
                    nc.vector.tensor_tensor(
                        w1f[:, _h0:_h1, :],
                        bnat[:, _h0:_h1, :],
                        cnat[:, _h0:_h1, None].to_broadcast(
                            (128, _h1 - _h0, 128)
                        ),
                        mybir.AluOpType.mult,
                    )

                # ---- windowed pooling via banded matmuls -------------------
                osb0 = posb.tile([128, 16, 128], F32, tag="o0")
                osb1 = posb.tile([128, 16, 128], F32, tag="o1")
                for wf, osb in ((w0f, osb0), (w1f, osb1)):
                    for g in range(4):
                        po = ppsM.tile([128, 4, 128], F32, tag="misc")
                        for q in range(4):
                            J = g * 4 + q
                            nc.tensor.matmul(
                                po[:, q, :],
                                lhsT=band0sb,
                                rhs=wf[:, J, :],
                                start=(q == 0),
                                stop=False,
                            )
                            nc.tensor.matmul(
                                po[:, q, :],
                                lhsT=band1sb,
                                rhs=wf[:, J + 1, :],
                                start=False,
                                stop=(q == 3),
                            )
                        nc.vector.tensor_copy(
                            osb[:, g * 4 : (g + 1) * 4, :], po
                        )
                nc.sync.dma_start(
                    out=o0[b].rearrange("(J p) h -> p J h", p=128), in_=osb0
                )
                nc.sync.dma_start(
                    out=o1[b].rearrange("(J p) h -> p J h", p=128), in_=osb1
                )

            # software pipeline: epilogue of batch b overlaps main of b+1
            emit_pm(0)
            for b in range(1, B):
                emit_pm(b)
                emit_epi(b - 1)
            emit_epi(B - 1)

    nc.compile()
    return nc


@functools.cache
def _module(b_per_core=B_PER_CORE):
    return _build(b_per_core)


def _prep_inputs(x0c: np.ndarray, x1c: np.ndarray):
    """Per-core host-side aux inputs. x0c/x1c: [B, S, H] float32."""
    B = x0c.shape[0]
    sq0 = np.einsum("bsh,bsh->bs", x0c, x0c).astype(np.float32)  # [B, S]
    sq0p = np.zeros((B, SPAD), np.float32)
    sq0p[:, :S] = sq0
    sq0n = sq0p.reshape(B, NT, 128).transpose(0, 2, 1).copy()  # [B, 128, NT]

    sq1 = np.einsum(
        "bsh,bsh->bs", x1c.astype(np.float64), x1c.astype(np.float64)
    )
    v = -0.5 * sq1
    hi = v.astype(ml_dtypes.bfloat16)
    lo = (v - hi.astype(np.float64)).astype(ml_dtypes.bfloat16)
    sq1hl = np.stack([hi, lo], axis=1)  # [B, 2, S] bf16
    return sq0n, sq1hl


def kernel(x0: np.ndarray, x1: np.ndarray):
    x0 = np.ascontiguousarray(np.asarray(x0, dtype=np.float32))
    x1 = np.ascontiguousarray(np.asarray(x1, dtype=np.float32))
    Bt = x0.shape[0]
    assert x0.shape == (Bt, 1, S, H), x0.shape
    bpc = Bt // N_CORES
    nc = _module(bpc)

    in_maps = []
    for c in range(N_CORES):
        x0c = np.ascontiguousarray(x0[c * bpc : (c + 1) * bpc, 0])
        x1c = np.ascontiguousarray(x1[c * bpc : (c + 1) * bpc, 0])
        sq0n, sq1hl = _prep_inputs(x0c, x1c)
        in_maps.append({"x0": x0c, "x1": x1c, "sq0n": sq0n, "sq1hl": sq1hl})

    res = run_bass_kernel_spmd(nc, in_maps, core_ids=list(range(N_CORES)))
    out0 = np.concatenate([r["o0"] for r in res.results], axis=0)
    out1 = np.concatenate([r["o1"] for r in res.results], axis=0)
    return (
        out0.reshape(Bt, 1, L_OUT, H),
        out1.reshape(Bt, 1, L_OUT, H),
    )


if __name__ == "__main__":
    inp = {
        "x0": np.random.randn(B_TOTAL, 1, S, H).astype(np.float32),
        "x1": np.random.randn(B_TOTAL, 1, S, H).astype(np.float32),
    }
    r0, r1 = kernel(**inp)
    print(r0.shape, r1.shape)

